# revision 1
# baseline (speedup 1.0000x reference)
"""Sequence-parallel self-attention kernel for 8 TRN2 NeuronCores.

Reference computation (N=8192, D=256, fp32):
    q = x @ WQ; k = x @ WK; v = x @ WV
    out = softmax(q @ k.T) @ v

Sharding: q-rows are split across 8 cores (1024 rows each); x is replicated
(host passes it pre-transposed as xT plus natural-layout x), so no
collectives are needed.

Per-core algebra (everything stays transposed so softmax's k-reduction is a
partition-axis ones-matmul and no on-chip transposes are needed):
    qT = WQ.T @ xT_local                      [256, 1024]
    M  = WK @ qT        (lhsT = WK.T)         [256, 1024]
    per k-chunk c (64 chunks of 128):
      scoresT = x_c @ M                       [128, 1024]   (= (q @ k.T).T chunk)
      expT    = exp(scoresT - 15)             (constant shift cancels in softmax)
      sums   += ones[128,1].T @ expT          [1, 1024]     (softmax denominator)
      UT     += x_c.T @ expT                  [256, 1024]   (= (attn_unnorm @ x).T)
    UTn  = UT * broadcast(1/sums)
    outT = WV.T @ UTn                         [256, 1024]   (= out.T, host transposes)

All matmuls run as float32r (full PE rate at free-dim >= 256, vs 4x slower
fp32). Every tensor feeding a matmul is declared float32r end-to-end (DRAM
inputs included) — the BIR verifier requires producers of fp32r-consumed
data to emit fp32r; numpy still sees plain float32 bytes.

PSUM budget (8 banks): UT 2x[128,1024]=4, sums 2x[1,512]=2, scoresT
double-buffer 2x[128,512]=2. Tail tiles reuse the same pool slots.
"""

import numpy as np

N, D, P = 8192, 256, 8
NL = N // P          # 1024 q-rows per core
KC = 128             # k-chunk size (contraction tile)
NCHUNK = N // KC     # 64
SB = 8               # k-chunks per DMA superblock
EXP_SHIFT = -15.0    # exp(s - 15): keeps ACT exp-table args in a good range

_CACHE = {}


def _build():
    import concourse.bacc as bacc
    import concourse.mybir as mybir
    import concourse.tile as tile

    f32 = mybir.dt.float32
    f32r = mybir.dt.float32r
    EXP = mybir.ActivationFunctionType.Exp

    nc = bacc.Bacc("TRN2", target_bir_lowering=False, debug=False,
                   enable_asserts=False)

    xT = nc.dram_tensor("xT", [D, N], f32r, kind="ExternalInput").ap()
    xn = nc.dram_tensor("xn", [N, D], f32r, kind="ExternalInput").ap()
    xTl = nc.dram_tensor("xTl", [D, NL], f32r, kind="ExternalInput").ap()
    wq = nc.dram_tensor("wq", [D, D], f32r, kind="ExternalInput").ap()
    wkt = nc.dram_tensor("wkt", [D, D], f32r, kind="ExternalInput").ap()
    wv = nc.dram_tensor("wv", [D, D], f32r, kind="ExternalInput").ap()
    onesd = nc.dram_tensor("onesd", [128, 128], f32r, kind="ExternalInput").ap()
    outT = nc.dram_tensor("outT", [D, NL], f32, kind="ExternalOutput").ap()

    with tile.TileContext(nc) as tc:
        with (
            tc.tile_pool(name="const", bufs=1) as cpool,
            tc.tile_pool(name="proj", bufs=1) as ppool,
            tc.tile_pool(name="xts", bufs=4) as xtpool,
            tc.tile_pool(name="xns", bufs=4) as xnpool,
            tc.tile_pool(name="expt", bufs=8) as epool,
            tc.tile_pool(name="tail", bufs=1) as tpool,
            tc.tile_pool(name="ps_scores", bufs=2, space="PSUM") as ps_s,
            tc.tile_pool(name="ps_ut", bufs=1, space="PSUM") as ps_ut,
            tc.tile_pool(name="ps_sums", bufs=1, space="PSUM") as ps_sum,
        ):
            # ---- constants / weights ----
            wq_t = [cpool.tile([128, D], f32r, tag=f"wq{h}", name=f"wq{h}") for h in range(2)]
            wkt_t = [cpool.tile([128, D], f32r, tag=f"wkt{h}", name=f"wkt{h}") for h in range(2)]
            wv_t = [cpool.tile([128, D], f32r, tag=f"wv{h}", name=f"wv{h}") for h in range(2)]
            xTl_t = [cpool.tile([128, NL], f32r, tag=f"xtl{h}", name=f"xtl{h}") for h in range(2)]
            ones_col = cpool.tile([128, 1], f32r, tag="ones_col", name="ones_col")
            ones_row = cpool.tile([1, 128], f32r, tag="ones_row", name="ones_row")
            bias_t = cpool.tile([128, 1], f32, tag="bias_t", name="bias_t")
            for h in range(2):
                nc.sync.dma_start(wq_t[h][:], wq[h * 128:(h + 1) * 128, :])
                nc.sync.dma_start(wkt_t[h][:], wkt[h * 128:(h + 1) * 128, :])
                nc.sync.dma_start(wv_t[h][:], wv[h * 128:(h + 1) * 128, :])
                nc.sync.dma_start(xTl_t[h][:], xTl[h * 128:(h + 1) * 128, :])
            nc.sync.dma_start(ones_col[:], onesd[:, 0:1])
            nc.sync.dma_start(ones_row[:], onesd[0:1, :])
            nc.vector.memset(bias_t[:], EXP_SHIFT)

            # ---- qT = WQ.T @ xT_local ; M = WK @ qT ----
            qT_t = [ppool.tile([128, NL], f32r, tag=f"qt{h}", name=f"qt{h}") for h in range(2)]
            m_t = [ppool.tile([128, NL], f32r, tag=f"m{h}", name=f"m{h}") for h in range(2)]
            for dst, lhs in ((qT_t, wq_t), (m_t, wkt_t)):
                src = xTl_t if dst is qT_t else qT_t
                for mh in range(2):
                    for nh in range(2):
                        pp = ps_s.tile([128, 512], f32, tag="scores", name="scores")
                        for kp in range(2):
                            nc.tensor.matmul(
                                pp[:],
                                lhs[kp][:, mh * 128:(mh + 1) * 128],
                                src[kp][:, nh * 512:(nh + 1) * 512],
                                start=(kp == 0), stop=(kp == 1),
                            )
                        nc.vector.tensor_copy(
                            dst[mh][:, nh * 512:(nh + 1) * 512], pp[:])

            # ---- persistent accumulators ----
            ut_ps = [ps_ut.tile([128, NL], f32, tag=f"ut{h}", name=f"ut{h}") for h in range(2)]
            sums_ps = [ps_sum.tile([1, 512], f32, tag=f"sums{h}", name=f"sums{h}")
                       for h in range(2)]

            # ---- main k-loop ----
            for sb in range(N // (KC * SB)):
                xt_t = [xtpool.tile([128, KC * SB], f32r, tag=f"xt{h}", name=f"xt{h}")
                        for h in range(2)]
                for h in range(2):
                    nc.sync.dma_start(
                        xt_t[h][:],
                        xT[h * 128:(h + 1) * 128,
                           sb * KC * SB:(sb + 1) * KC * SB])
                xn_t = xnpool.tile([128, SB, D], f32r, tag="xn", name="xn")
                nc.sync.dma_start(
                    xn_t[:],
                    xn[sb * KC * SB:(sb + 1) * KC * SB, :]
                    .rearrange("(a p) d -> p a d", p=128))

                for j in range(SB):
                    c = sb * SB + j
                    first, last = (c == 0), (c == NCHUNK - 1)
                    exps = []
                    for qh in range(2):
                        sp = ps_s.tile([128, 512], f32, tag="scores", name="scores")
                        for kp in range(2):
                            nc.tensor.matmul(
                                sp[:],
                                xt_t[kp][:, j * KC:(j + 1) * KC],
                                m_t[kp][:, qh * 512:(qh + 1) * 512],
                                start=(kp == 0), stop=(kp == 1),
                            )
                        et = epool.tile([128, 512], f32r, tag="expt", name="expt")
                        nc.scalar.activation(et[:], sp[:], EXP, bias=bias_t[:])
                        exps.append(et)
                    for qh in range(2):
                        et = exps[qh]
                        nc.tensor.matmul(
                            sums_ps[qh][:], ones_col[:], et[:],
                            start=first, stop=last)
                        for dh in range(2):
                            nc.tensor.matmul(
                                ut_ps[dh][:, qh * 512:(qh + 1) * 512],
                                xn_t[:, j, dh * 128:(dh + 1) * 128],
                                et[:],
                                start=first, stop=last)

            # ---- tail: softmax normalize + WV projection ----
            sums_sb = tpool.tile([1, NL], f32, tag="sums_sb", name="sums_sb")
            for qh in range(2):
                nc.vector.tensor_copy(
                    sums_sb[:, qh * 512:(qh + 1) * 512], sums_ps[qh][:])
            recip_sb = tpool.tile([1, NL], f32r, tag="recip_sb", name="recip_sb")
            with nc.allow_low_precision(reason="f32r is 4-byte, same mantissa path"):
                nc.vector.reciprocal(recip_sb[:], sums_sb[:])

            rb_sb = tpool.tile([128, NL], f32, tag="rb_sb", name="rb_sb")
            for qh in range(2):
                rp = ps_s.tile([128, 512], f32, tag="scores", name="scores")
                nc.tensor.matmul(
                    rp[:], ones_row[:],
                    recip_sb[:, qh * 512:(qh + 1) * 512],
                    start=True, stop=True)
                nc.vector.tensor_copy(rb_sb[:, qh * 512:(qh + 1) * 512], rp[:])

            utn_sb = [tpool.tile([128, NL], f32r, tag=f"utn{h}", name=f"utn{h}")
                      for h in range(2)]
            for dh in range(2):
                nc.vector.tensor_mul(utn_sb[dh][:], ut_ps[dh][:], rb_sb[:])

            o_sb = [tpool.tile([128, NL], f32, tag=f"osb{h}", name=f"osb{h}") for h in range(2)]
            for mh in range(2):
                op = ps_ut.tile([128, NL], f32, tag=f"ut{mh}", name=f"ut{mh}")
                for nh in range(2):
                    for kp in range(2):
                        nc.tensor.matmul(
                            op[:, nh * 512:(nh + 1) * 512],
                            wv_t[kp][:, mh * 128:(mh + 1) * 128],
                            utn_sb[kp][:, nh * 512:(nh + 1) * 512],
                            start=(kp == 0), stop=(kp == 1),
                        )
                nc.vector.tensor_copy(o_sb[mh][:], op[:])
                nc.sync.dma_start(outT[mh * 128:(mh + 1) * 128, :], o_sb[mh][:])

    nc.compile()
    return nc


def _get_nc():
    if "nc" not in _CACHE:
        _CACHE["nc"] = _build()
    return _CACHE["nc"]


def kernel(input, WQ, WK, WV):
    from concourse import bass_utils

    x = np.ascontiguousarray(input, dtype=np.float32)
    xT = np.ascontiguousarray(x.T)
    wq = np.ascontiguousarray(WQ, dtype=np.float32)
    wkt = np.ascontiguousarray(np.asarray(WK, dtype=np.float32).T)
    wv = np.ascontiguousarray(WV, dtype=np.float32)

    nc = _get_nc()
    in_maps = []
    for c in range(P):
        in_maps.append({
            "xT": xT,
            "xn": x,
            "xTl": np.ascontiguousarray(xT[:, c * NL:(c + 1) * NL]),
            "wq": wq,
            "wkt": wkt,
            "wv": wv,
            "onesd": np.ones((128, 128), dtype=np.float32),
        })
    res = bass_utils.run_bass_kernel_spmd(nc, in_maps, core_ids=list(range(P)))
    out = np.empty((N, D), dtype=np.float32)
    for c in range(P):
        out[c * NL:(c + 1) * NL, :] = res.results[c]["outT"].T
    return out

